# revision 6
# baseline (speedup 1.0000x reference)
"""Trainium2 Bass kernel for nn_ConceptIntergation (histogram_binning).

Reference computation:
    counts[b,s,n] = sum_k one_hot(concepts[b,s,k], 129)[..., n]  (n < 128; 128 = padding)
    out[b,s,n,d]  = counts[b,s,n] * emb_table[n,d]

Strategy (data-parallel over batch, 8 cores; transposed class-major layout):
  - Each core handles B_LOC=8 batches -> R=1600 (b,s) rows. The kernel is
    HBM-store bound (26.2 MB bf16 per core), and each of the 16 SDMA
    engines is port-limited to ~27 GB/s, so everything is organized to
    keep all 16 engines saturated from as early as possible.
  - Device layout puts the CONCEPT CLASS n on the partition axis:
      countsT[n, r] = sum_k (concepts[r,k] == n)
      out_d[n, r]   = emb[n, d] * countsT[n, r]
    With n on partitions, emb[:, d] is a per-partition scalar, so the big
    multiply runs as DVE tensor_scalar in the 4x perf mode (~630ns per
    [128,1600] bf16 slice); the Scalar engine (activation Copy with
    per-partition scale AP) computes 16 of the 64 d-slices concurrently.
    The histogram is 4 tensor_scalar is_equal ops (one per k, 4x mode)
    with in-place tensor_tensor accumulation, interleaved with the 4
    k-major index chunk loads.
  - SDMA engine 15 is ~20% slower than the others (known trn2 trait) and
    serves partitions {92..95, 124..127}. Those 8 classes' d in [52,64)
    slices are computed redundantly on donor partitions 0..15 (engines
    0/2/4/6) via a counts row-copy + per-partition emb2 scalars, stored
    to a side tensor, and the corresponding main stores exclude the slow
    partitions - cutting engine 15's bytes by ~25%.
  - Stores issue on both HWDGE rings (Sync ring for DVE groups, ACT ring
    for ScalarE groups) in 4-d-slice batches (~1.6 MB each). Host
    transposes shards to [r, n, d] and upcasts bf16->f32 exactly.
"""

import numpy as np
import ml_dtypes

import concourse.bass as bass
import concourse.mybir as mybir
from concourse import bacc
from concourse.tile import TileContext
from concourse.bass_utils import run_bass_kernel_spmd

B, S, K = 64, 200, 4
N, D = 128, 64
NCORES = 8
B_LOC = B // NCORES             # 8
R = B_LOC * S                   # 1600 (b,s) rows per core
P = 128
RK = K * R                      # 6400
OUTW = D * R                    # 102400

GD = 4                          # d-slices per store group
_DVE_D = [(d, d + GD) for d in range(0, 48, GD)]     # DVE owns d 0..47
_SCE_D = [(d, d + GD) for d in range(48, 64, GD)]    # ScalarE owns d 48..63

# slow-engine offload: SDMA engine 15 serves these partitions
SLOW_LO = (92, 96)
SLOW_HI = (124, 128)
SLOW_CLASSES = list(range(*SLOW_LO)) + list(range(*SLOW_HI))  # 8 classes
OFF_D0 = 52                     # offloaded d range [52, 64)
OFF_F = 64 - OFF_D0             # 12 d per slow class
NDON = 16                       # donor partitions 0..15 (engines 0/2/4/6)
FSL = len(SLOW_CLASSES) * OFF_F // NDON   # 6 slices per donor

BF16 = mybir.dt.bfloat16
F32 = mybir.dt.float32

_NC_CACHE = {}


def _build_nc():
    nc = bacc.Bacc()
    idxb = nc.declare_dram_parameter("idxb", [P, RK], BF16, isOutput=False)
    pe = nc.declare_dram_parameter("pe", [P, 1 + D], F32, isOutput=False)
    emb2 = nc.declare_dram_parameter("emb2", [NDON, FSL], F32, isOutput=False)
    out = nc.declare_dram_parameter("out", [P, OUTW], BF16, isOutput=True)
    out2 = nc.declare_dram_parameter("out2", [NDON, FSL * R], BF16, isOutput=True)

    mult = mybir.AluOpType.mult
    add = mybir.AluOpType.add
    is_eq = mybir.AluOpType.is_equal

    with TileContext(nc) as tc:
        with (
            tc.tile_pool(name="const", bufs=1) as cpool,
            tc.tile_pool(name="vout", bufs=6) as vpool,
            tc.tile_pool(name="sout", bufs=2) as spool,
        ):
            pe_sb = cpool.tile([P, 1 + D], F32)
            nc.sync.dma_start(out=pe_sb, in_=pe[:, :])
            pcol = pe_sb[:, 0:1]
            # k-major replicated indices; 4 chunk loads so the histogram
            # chain starts as soon as chunk 0 lands.
            idx_sb = cpool.tile([P, RK], BF16)
            for k in range(K):
                nc.sync.dma_start(
                    out=idx_sb[:, k * R : (k + 1) * R],
                    in_=idxb[:, k * R : (k + 1) * R],
                )
            emb2_sb = cpool.tile([NDON, FSL], F32)
            nc.sync.dma_start(out=emb2_sb, in_=emb2[:, :])

            # warm the ScalarE activation table during the input DMAs
            warm = cpool.tile([P, 1], F32)
            nc.scalar.copy(out=warm, in_=pcol)

            # countsT[n, r] = sum_k (idx[r,k] == n); chained so each op only
            # needs its own chunk: TS k0, TS k1, TT +=, TS k2, TT +=, ...
            counts = cpool.tile([P, R], BF16)
            ck = cpool.tile([P, R], BF16)

            def ts_eq(dst, k):
                nc.vector.tensor_scalar(
                    out=dst, in0=idx_sb[:, k * R : (k + 1) * R],
                    scalar1=pcol, scalar2=None, op0=is_eq,
                )

            ts_eq(counts, 0)
            ts_eq(ck, 1)
            nc.vector.tensor_tensor(out=counts, in0=counts, in1=ck, op=add)
            ts_eq(ck, 2)
            nc.vector.tensor_tensor(out=counts, in0=counts, in1=ck, op=add)
            ts_eq(ck, 3)
            nc.vector.tensor_tensor(out=counts, in0=counts, in1=ck, op=add)

            def vmul(dst, dd):
                nc.vector.tensor_scalar(
                    out=dst, in0=counts,
                    scalar1=pe_sb[:, 1 + dd : 2 + dd], scalar2=None, op0=mult,
                )

            def emit_vgroup(d0, d1):
                ob = vpool.tile([P, (d1 - d0) * R], BF16, tag="vob")
                for i in range(d1 - d0):
                    vmul(ob[:, i * R : (i + 1) * R], d0 + i)
                nc.sync.dma_start(out=out[:, d0 * R : d1 * R], in_=ob)

            def emit_sgroup(d0, d1):
                ob = spool.tile([P, (d1 - d0) * R], BF16, tag="sob")
                for i in range(d1 - d0):
                    nc.scalar.mul(
                        out=ob[:, i * R : (i + 1) * R], in_=counts,
                        mul=pe_sb[:, 1 + (d0 + i) : 2 + (d0 + i)],
                    )
                if d0 >= OFF_D0:
                    # slow partitions' data for these d comes from out2
                    nc.scalar.dma_start(
                        out=out[0 : SLOW_LO[0], d0 * R : d1 * R],
                        in_=ob[0 : SLOW_LO[0], :],
                    )
                    nc.scalar.dma_start(
                        out=out[SLOW_LO[1] : SLOW_HI[0], d0 * R : d1 * R],
                        in_=ob[SLOW_LO[1] : SLOW_HI[0], :],
                    )
                else:
                    nc.scalar.dma_start(out=out[:, d0 * R : d1 * R], in_=ob)

            # first two DVE groups out fast
            emit_vgroup(*_DVE_D[0])
            emit_vgroup(*_DVE_D[1])

            # ScalarE groups (independent engine, runs concurrently)
            for d0, d1 in _SCE_D:
                emit_sgroup(d0, d1)

            # offload: copy slow classes' counts rows to donor partitions,
            # then 6 [16,1600] muls with per-donor emb2 scalars
            counts2 = cpool.tile([NDON, R], BF16)
            nc.sync.dma_start(out=counts2[0:4, :], in_=counts[SLOW_LO[0] : SLOW_LO[1], :])
            nc.sync.dma_start(out=counts2[4:8, :], in_=counts[SLOW_HI[0] : SLOW_HI[1], :])
            nc.sync.dma_start(out=counts2[8:12, :], in_=counts[SLOW_LO[0] : SLOW_LO[1], :])
            nc.sync.dma_start(out=counts2[12:16, :], in_=counts[SLOW_HI[0] : SLOW_HI[1], :])
            out2t = cpool.tile([NDON, FSL * R], BF16)
            for j in range(FSL):
                nc.vector.tensor_scalar(
                    out=out2t[:, j * R : (j + 1) * R], in0=counts2,
                    scalar1=emb2_sb[:, j : j + 1], scalar2=None, op0=mult,
                )
            nc.sync.dma_start(out=out2[:, :], in_=out2t)

            for d0, d1 in _DVE_D[2:]:
                emit_vgroup(d0, d1)

    nc.finalize()
    return nc


def _get_nc():
    if "nc" not in _NC_CACHE:
        _NC_CACHE["nc"] = _build_nc()
    return _NC_CACHE["nc"]


def _prepare_in_maps(concepts, emb_table):
    concepts = np.asarray(concepts)
    emb = np.asarray(emb_table, dtype=np.float32)

    # per-core k-major index shards replicated across the 128 partitions
    conc = concepts.reshape(NCORES, R, K)
    idx_kmaj = np.ascontiguousarray(conc.transpose(0, 2, 1)).reshape(NCORES, 1, RK)
    idx_dev = np.ascontiguousarray(
        np.broadcast_to(idx_kmaj.astype(ml_dtypes.bfloat16), (NCORES, P, RK))
    )

    pe = np.empty((P, 1 + D), dtype=np.float32)
    pe[:, 0] = np.arange(P, dtype=np.float32)
    pe[:, 1:] = emb

    # donor scalars: donor i -> class SLOW_CLASSES[i%8], d = OFF_D0 + FSL*(i//8) + j
    emb2 = np.empty((NDON, FSL), dtype=np.float32)
    for i in range(NDON):
        c = SLOW_CLASSES[i % 8]
        base = OFF_D0 + FSL * (i // 8)
        emb2[i, :] = emb[c, base : base + FSL]

    return [
        {"idxb": idx_dev[i], "pe": pe, "emb2": emb2}
        for i in range(NCORES)
    ]


def _run(concepts, emb_table, **spmd_kwargs):
    nc = _get_nc()
    in_maps = _prepare_in_maps(concepts, emb_table)
    res = run_bass_kernel_spmd(nc, in_maps, core_ids=list(range(NCORES)), **spmd_kwargs)
    # shards are [128(n), 64(d)*1600(r)] bf16; -> [r, n, d], upcast exactly
    u16 = np.stack(
        [np.asarray(res.results[i]["out"]).view(np.uint16) for i in range(NCORES)]
    ).reshape(NCORES, N, D, R)
    # patch the slow classes' offloaded d range from out2
    for i in range(NCORES):
        o2 = np.asarray(res.results[i]["out2"]).view(np.uint16).reshape(NDON, FSL, R)
        for don in range(NDON):
            c = SLOW_CLASSES[don % 8]
            base = OFF_D0 + FSL * (don // 8)
            u16[i, c, base : base + FSL, :] = o2[don]
    u16 = u16.transpose(0, 3, 1, 2)  # -> [core, r, n, d]
    f32 = (u16.astype(np.uint32) << 16).view(np.float32)
    out = f32.reshape(B, S, N, D)
    return out, res


def kernel(concepts, emb_table):
    out, _ = _run(concepts, emb_table)
    return out


# revision 7
# speedup vs baseline: 1.4850x; 1.4850x over previous
"""Trainium2 Bass kernel for nn_ConceptIntergation (histogram_binning).

Reference computation:
    counts[b,s,n] = sum_k one_hot(concepts[b,s,k], 129)[..., n]  (n < 128; 128 = padding)
    out[b,s,n,d]  = counts[b,s,n] * emb_table[n,d]

Strategy (data-parallel over batch, 8 cores; transposed class-major layout):
  - Each core handles B_LOC=8 batches -> R=1600 (b,s) rows. The kernel is
    HBM-store bound (26.2 MB bf16 per core), and each of the 16 SDMA
    engines is port-limited to ~27 GB/s, so everything is organized to
    keep all 16 engines saturated from as early as possible.
  - Device layout puts the CONCEPT CLASS n on the partition axis:
      countsT[n, r] = sum_k (concepts[r,k] == n)
      out_d[n, r]   = emb[n, d] * countsT[n, r]
    With n on partitions, emb[:, d] is a per-partition scalar, so the big
    multiply runs as DVE tensor_scalar in the 4x perf mode (~630ns per
    [128,1600] bf16 slice); the Scalar engine (activation Copy with
    per-partition scale AP) computes 16 of the 64 d-slices concurrently.
    The histogram is 4 tensor_scalar is_equal ops (one per k, 4x mode)
    with in-place tensor_tensor accumulation, interleaved with the 4
    k-major index chunk loads so the chain finishes right after the last
    chunk lands.
  - Stores issue on both HWDGE rings (Sync ring for DVE groups, ACT ring
    for ScalarE groups) in multi-d-slice batches, with small first groups
    so the SDMA engines start early. Host transposes shards to [r, n, d]
    and upcasts bf16->f32 with an exact bit shift.
"""

import numpy as np
import ml_dtypes

import concourse.bass as bass
import concourse.mybir as mybir
from concourse import bacc
from concourse.tile import TileContext
from concourse.bass_utils import run_bass_kernel_spmd

B, S, K = 64, 200, 4
N, D = 128, 64
NCORES = 8
B_LOC = B // NCORES             # 8
R = B_LOC * S                   # 1600 (b,s) rows per core
P = 128
RK = K * R                      # 6400
OUTW = D * R                    # 102400

# d-slice store groups: DVE owns d 0..47 (Sync ring), ScalarE d 48..63
# (ACT ring); small first groups get the SDMA engines going early.
_DVE_D = [(0, 2), (2, 4), (4, 8), (8, 12), (12, 16), (16, 20), (20, 24),
          (24, 28), (28, 32), (32, 36), (36, 40), (40, 44), (44, 48)]
_SCE_D = [(48, 50), (50, 52), (52, 56), (56, 60), (60, 64)]

BF16 = mybir.dt.bfloat16
F32 = mybir.dt.float32

_NC_CACHE = {}


def _build_nc():
    nc = bacc.Bacc()
    idxb = nc.declare_dram_parameter("idxb", [P, RK], BF16, isOutput=False)
    pe = nc.declare_dram_parameter("pe", [P, 1 + D], F32, isOutput=False)
    out = nc.declare_dram_parameter("out", [P, OUTW], BF16, isOutput=True)

    mult = mybir.AluOpType.mult
    add = mybir.AluOpType.add
    is_eq = mybir.AluOpType.is_equal

    with TileContext(nc) as tc:
        with (
            tc.tile_pool(name="const", bufs=1) as cpool,
            tc.tile_pool(name="vout", bufs=6) as vpool,
            tc.tile_pool(name="sout", bufs=3) as spool,
        ):
            pe_sb = cpool.tile([P, 1 + D], F32)
            nc.sync.dma_start(out=pe_sb, in_=pe[:, :])
            pcol = pe_sb[:, 0:1]
            # k-major replicated indices; 4 chunk loads so the histogram
            # chain starts as soon as chunk 0 lands.
            idx_sb = cpool.tile([P, RK], BF16)
            for k in range(K):
                nc.sync.dma_start(
                    out=idx_sb[:, k * R : (k + 1) * R],
                    in_=idxb[:, k * R : (k + 1) * R],
                )

            # warm the ScalarE activation table during the input DMAs
            warm = cpool.tile([P, 1], F32)
            nc.scalar.copy(out=warm, in_=pcol)

            # countsT[n, r] = sum_k (idx[r,k] == n); chained so each op only
            # needs its own chunk: TS k0, TS k1, TT +=, TS k2, TT +=, ...
            counts = cpool.tile([P, R], BF16)
            ck = cpool.tile([P, R], BF16)

            def ts_eq(dst, k):
                nc.vector.tensor_scalar(
                    out=dst, in0=idx_sb[:, k * R : (k + 1) * R],
                    scalar1=pcol, scalar2=None, op0=is_eq,
                )

            ts_eq(counts, 0)
            ts_eq(ck, 1)
            nc.vector.tensor_tensor(out=counts, in0=counts, in1=ck, op=add)
            ts_eq(ck, 2)
            nc.vector.tensor_tensor(out=counts, in0=counts, in1=ck, op=add)
            ts_eq(ck, 3)
            nc.vector.tensor_tensor(out=counts, in0=counts, in1=ck, op=add)

            def emit_vgroup(d0, d1):
                ob = vpool.tile([P, (d1 - d0) * R], BF16, tag="vob")
                for i in range(d1 - d0):
                    nc.vector.tensor_scalar(
                        out=ob[:, i * R : (i + 1) * R], in0=counts,
                        scalar1=pe_sb[:, 1 + (d0 + i) : 2 + (d0 + i)],
                        scalar2=None, op0=mult,
                    )
                nc.sync.dma_start(out=out[:, d0 * R : d1 * R], in_=ob)

            def emit_sgroup(d0, d1):
                ob = spool.tile([P, (d1 - d0) * R], BF16, tag="sob")
                for i in range(d1 - d0):
                    nc.scalar.mul(
                        out=ob[:, i * R : (i + 1) * R], in_=counts,
                        mul=pe_sb[:, 1 + (d0 + i) : 2 + (d0 + i)],
                    )
                nc.scalar.dma_start(out=out[:, d0 * R : d1 * R], in_=ob)

            for d0, d1 in _SCE_D:
                emit_sgroup(d0, d1)
            for d0, d1 in _DVE_D:
                emit_vgroup(d0, d1)

    nc.finalize()
    return nc


def _get_nc():
    if "nc" not in _NC_CACHE:
        _NC_CACHE["nc"] = _build_nc()
    return _NC_CACHE["nc"]


def _prepare_in_maps(concepts, emb_table):
    concepts = np.asarray(concepts)
    emb = np.asarray(emb_table, dtype=np.float32)

    # per-core k-major index shards replicated across the 128 partitions
    conc = concepts.reshape(NCORES, R, K)
    idx_kmaj = np.ascontiguousarray(conc.transpose(0, 2, 1)).reshape(NCORES, 1, RK)
    idx_dev = np.ascontiguousarray(
        np.broadcast_to(idx_kmaj.astype(ml_dtypes.bfloat16), (NCORES, P, RK))
    )

    pe = np.empty((P, 1 + D), dtype=np.float32)
    pe[:, 0] = np.arange(P, dtype=np.float32)
    pe[:, 1:] = emb

    return [{"idxb": idx_dev[i], "pe": pe} for i in range(NCORES)]


def _run(concepts, emb_table, **spmd_kwargs):
    nc = _get_nc()
    in_maps = _prepare_in_maps(concepts, emb_table)
    res = run_bass_kernel_spmd(nc, in_maps, core_ids=list(range(NCORES)), **spmd_kwargs)
    # shards are [128(n), 64(d)*1600(r)] bf16; -> [r, n, d], upcast exactly
    u16 = np.stack(
        [np.asarray(res.results[i]["out"]).view(np.uint16) for i in range(NCORES)]
    ).reshape(NCORES, N, D, R)
    u16 = u16.transpose(0, 3, 1, 2)  # -> [core, r, n, d]
    f32 = (u16.astype(np.uint32) << 16).view(np.float32)
    out = f32.reshape(B, S, N, D)
    return out, res


def kernel(concepts, emb_table):
    out, _ = _run(concepts, emb_table)
    return out
